# revision 9
# baseline (speedup 1.0000x reference)
"""AFT local-attention kernel for Trainium2 (8 NeuronCores).

Math (exactly equivalent to the reference in exact arithmetic):
  reference computes  y = sigmoid(q) * num / den  with
    num = einsum('ij,jbd->ibd', exp(pb*mask - max_pb), exp(k - max_key) * v)
    den = einsum('ij,jbd->ibd', exp(pb*mask - max_pb), exp(k - max_key))
  The row factor exp(-max_pb[i]) and the per-(b,d) factor exp(bk - max_key)
  are constant along the contraction axis j, so they cancel in num/den.
  Outside the |i-j| < 64 band, exp(pb*mask) == 1, so with
    E[i,j] = (exp(pb[i,j]) - 1) * band_mask[i,j]      (banded, width 127)
    S[b,d] = sum_j ek[j,b,d]*v[j,b,d],  Z[b,d] = sum_j ek[j,b,d]
    ek     = exp(key @ Wk.T)            (bk cancels too)
  we get  num = E @ (ek*v) + S,  den = E @ ek + Z.

Sharding: core c -> batch b=c//2, T-half h=c%2 (2048 queries each).
k/v are loaded with a 64-row halo (zero-padded at the global edges; the
band term is zero there because pb is zero-padded, and the halo rows are
excluded from the local S/Z partial sums). S/Z are all-reduced over core
pairs [2b, 2b+1].
"""

import numpy as np
from contextlib import ExitStack

import concourse.bass as bass
import concourse.tile as tile
from concourse import bacc, mybir
from concourse.bass import ts, ds
from concourse.bass_utils import run_bass_kernel_spmd
from concourse.masks import make_identity

T = 4096
B = 4
D = 512
WIN = 64
TH = T // 2          # tokens per core (query rows)
NIT = TH // 128      # 16 i-tiles
KV = TH + 2 * WIN    # 2176 k/v rows incl. halo
KCH = KV // 128      # 17 k/v chunks

F32 = mybir.dt.float32
BF16 = mybir.dt.bfloat16
AF = mybir.ActivationFunctionType


def build():
    nc = bacc.Bacc("TRN2", target_bir_lowering=False, num_devices=8)

    q_d = nc.dram_tensor("q", [TH, D], F32, kind="ExternalInput")
    k_d = nc.dram_tensor("k", [KV, D], F32, kind="ExternalInput")
    v_d = nc.dram_tensor("v", [KV, D], F32, kind="ExternalInput")
    pb_d = nc.dram_tensor("pb", [NIT * 128, 256], F32, kind="ExternalInput")
    w_d = {
        nm: nc.dram_tensor(nm, [D, D], F32, kind="ExternalInput")
        for nm in ("wq", "wk", "wv", "wo")
    }
    b_d = {
        nm: nc.dram_tensor(nm, [1, D], F32, kind="ExternalInput")
        for nm in ("bq", "bv", "bo")
    }
    szm_d = nc.dram_tensor("szmask", [128, 3], F32, kind="ExternalInput")
    out_d = nc.dram_tensor("out", [TH, D], F32, kind="ExternalOutput")

    with tile.TileContext(nc) as tc, ExitStack() as ctx:
        const = ctx.enter_context(tc.tile_pool(name="const", bufs=1))
        big = ctx.enter_context(tc.tile_pool(name="big", bufs=1))

        ident = const.tile([128, 128], BF16)
        make_identity(nc, ident[:])

        ones_row = const.tile([1, 128], BF16)
        nc.vector.memset(ones_row[:], 1.0)

        szm = const.tile([128, 3], BF16)
        nc.gpsimd.dma_start(szm[:], szm_d[:, :])

        w_sb = {}
        for nm in ("wq", "wk", "wv", "wo"):
            wt = const.tile([128, 4, D], BF16, tag=f"w_{nm}")
            nc.gpsimd.dma_start(wt[:], w_d[nm][:, :].rearrange("(c p) n -> p c n", p=128))
            w_sb[nm] = wt
        b_sb = {}
        for nm in ("bq", "bv", "bo"):
            bt = const.tile([1, D], BF16, tag=f"b_{nm}")
            nc.gpsimd.dma_start(bt[:], b_d[nm][:, :])
            b_sb[nm] = bt

        # natural-layout (token-major) staging for the three inputs
        k_nat = big.tile([128, KCH, D], BF16)
        v_nat = big.tile([128, KCH, D], BF16)
        q_nat = big.tile([128, NIT, D], BF16)
        pb_nat = big.tile([128, NIT, 256], BF16)
        for c0 in range(0, KCH, 5):
            cn = min(5, KCH - c0)
            nc.gpsimd.dma_start(
                k_nat[:, c0 : c0 + cn, :],
                k_d[:, :].rearrange("(c p) n -> p c n", p=128)[:, c0 : c0 + cn, :],
            )
            nc.gpsimd.dma_start(
                v_nat[:, c0 : c0 + cn, :],
                v_d[:, :].rearrange("(c p) n -> p c n", p=128)[:, c0 : c0 + cn, :],
            )
        for c0 in range(0, NIT, 4):
            nc.gpsimd.dma_start(
                q_nat[:, c0 : c0 + 4, :],
                q_d[:, :].rearrange("(c p) n -> p c n", p=128)[:, c0 : c0 + 4, :],
            )
        nc.gpsimd.dma_start(
            pb_nat[:], pb_d[:, :].rearrange("(c p) n -> p c n", p=128)
        )

        ek_big = big.tile([128, KCH * D], BF16)
        ekv_big = big.tile([128, KCH * D], BF16)
        et_big = big.tile([128, NIT * 256], BF16)

        lhsT_p = ctx.enter_context(tc.tile_pool(name="lhsT", bufs=3))
        tp_ps = ctx.enter_context(tc.tile_pool(name="tpps", bufs=2, space="PSUM"))
        pj_ps = ctx.enter_context(tc.tile_pool(name="pjps", bufs=2, space="PSUM"))
        work = ctx.enter_context(tc.tile_pool(name="work", bufs=3))
        dram = ctx.enter_context(tc.tile_pool(name="dram", bufs=1, space="DRAM"))

        def transpose_chunk(src_ap, nq, tag):
            """PE-transpose a [128, nq*128] natural chunk -> [128, nq*128] where
            quarter j holds src[:, j*128:(j+1)*128].T; returns an SBUF bf16 tile."""
            ps = tp_ps.tile([128, nq * 128], BF16, tag="tp")
            for j in range(nq):
                nc.tensor.transpose(ps[:, ts(j, 128)], src_ap[:, ts(j, 128)], ident[:])
            sb = lhsT_p.tile([128, nq * 128], BF16, tag=tag)
            nc.vector.tensor_copy(sb[:], ps[:])
            return sb

        # ---- phase 1: k/v projections, ek/ekv, S/Z partial sums ----
        with tc.tile_pool(name="szps", bufs=1, space="PSUM") as szp:
            z_ps = szp.tile([1, D], F32, tag="z")
            s_ps = szp.tile([1, D], F32, tag="s")
            for c in range(KCH):
                sel = 0 if c == 0 else (2 if c == KCH - 1 else 1)
                kT = transpose_chunk(k_nat[:, c, :], 4, "xT")
                kp = pj_ps.tile([128, D], F32, tag="pj")
                for j in range(4):
                    nc.tensor.matmul(
                        kp[:], kT[:, ts(j, 128)], w_sb["wk"][:, j, :],
                        start=(j == 0), stop=(j == 3),
                    )
                nc.scalar.activation(ek_big[:, ts(c, D)], kp[:], AF.Exp)

                vT = transpose_chunk(v_nat[:, c, :], 4, "xT")
                vp = pj_ps.tile([128, D], F32, tag="pj")
                for j in range(4):
                    nc.tensor.matmul(
                        vp[:], vT[:, ts(j, 128)], w_sb["wv"][:, j, :],
                        start=(j == 0), stop=False,
                    )
                nc.tensor.matmul(
                    vp[:], ones_row[:1, :], b_sb["bv"][:1, :], start=False, stop=True
                )
                nc.vector.tensor_mul(ekv_big[:, ts(c, D)], ek_big[:, ts(c, D)], vp[:])

                nc.tensor.matmul(
                    z_ps[:], szm[:, sel : sel + 1], ek_big[:, ts(c, D)],
                    start=(c == 0), stop=(c == KCH - 1), skip_group_check=True,
                )
                nc.tensor.matmul(
                    s_ps[:], szm[:, sel : sel + 1], ekv_big[:, ts(c, D)],
                    start=(c == 0), stop=(c == KCH - 1), skip_group_check=True,
                )

            # ---- phase 2: S/Z all-reduce over the core pair ----
            sz_sb = const.tile([1, 2 * D], F32)
            nc.vector.tensor_copy(sz_sb[:, 0:D], z_ps[:])
            nc.vector.tensor_copy(sz_sb[:, D : 2 * D], s_ps[:])

        cc_in = dram.tile([1, 2 * D], F32)
        cc_out = dram.tile([1, 2 * D], F32)  # Shared addr_space unsupported for 2-core groups
        nc.sync.dma_start(cc_in[:], sz_sb[:])
        nc.gpsimd.collective_compute(
            "AllReduce",
            mybir.AluOpType.add,
            replica_groups=[[0, 1], [2, 3], [4, 5], [6, 7]],
            ins=[cc_in[:].opt()],
            outs=[cc_out[:].opt()],
        )
        szr = const.tile([1, 2 * D], F32)
        nc.sync.dma_start(szr[:], cc_out[:])
        # split into bf16 hi+lo rows so the K=1 fold matmuls keep ~fp32 accuracy
        zs_hi = const.tile([1, 2 * D], BF16)
        zs_lo = const.tile([1, 2 * D], BF16)
        nc.vector.tensor_copy(zs_hi[:], szr[:])
        nc.vector.tensor_tensor(zs_lo[:], szr[:], zs_hi[:], mybir.AluOpType.subtract)
        z_hi, z_lo = zs_hi[:, 0:D], zs_lo[:, 0:D]
        s_hi, s_lo = zs_hi[:, D : 2 * D], zs_lo[:, D : 2 * D]

        # ---- phase 3: E.T tiles from pos_bias band slices ----
        for it in range(NIT):
            pps = tp_ps.tile([128, 256], BF16, tag="tp")
            for j in range(2):
                nc.tensor.transpose(
                    pps[:, ts(j, 128)], pb_nat[:, it, ts(j, 128)], ident[:]
                )
            pe_f = work.tile([128, 256], F32, tag="pexp")
            nc.scalar.activation(pe_f[:], pps[:], AF.Exp)
            nc.vector.tensor_scalar_sub(et_big[:, ts(it, 256)], pe_f[:], 1.0)

        # ---- phase 4: per i-tile q/sigmoid, band einsum, combine, out proj ----
        with tc.tile_pool(name="einps", bufs=2, space="PSUM") as einp:
            for it in range(NIT):
                qT = transpose_chunk(q_nat[:, it, :], 4, "xT")
                qp = pj_ps.tile([128, D], F32, tag="pj")
                for j in range(4):
                    nc.tensor.matmul(
                        qp[:], qT[:, ts(j, 128)], w_sb["wq"][:, j, :],
                        start=(j == 0), stop=False,
                    )
                nc.tensor.matmul(
                    qp[:], ones_row[:1, :], b_sb["bq"][:1, :], start=False, stop=True
                )
                sig = work.tile([128, D], F32, tag="sig")
                nc.scalar.activation(sig[:], qp[:], AF.Sigmoid)

                bn = einp.tile([128, D], F32, tag="bn")
                bd = einp.tile([128, D], F32, tag="bd")
                e0 = et_big[:, ds(it * 256, 128)]
                e1 = et_big[:, ds(it * 256 + 128, 128)]
                nc.tensor.matmul(bd[:], e0, ek_big[:, ts(it, D)], start=True, stop=False)
                nc.tensor.matmul(bd[:], e1, ek_big[:, ts(it + 1, D)], start=False, stop=False)
                nc.tensor.matmul(bd[:], ones_row[:1, :], z_hi, start=False, stop=False)
                nc.tensor.matmul(bd[:], ones_row[:1, :], z_lo, start=False, stop=True)
                nc.tensor.matmul(bn[:], e0, ekv_big[:, ts(it, D)], start=True, stop=False)
                nc.tensor.matmul(bn[:], e1, ekv_big[:, ts(it + 1, D)], start=False, stop=False)
                nc.tensor.matmul(bn[:], ones_row[:1, :], s_hi, start=False, stop=False)
                nc.tensor.matmul(bn[:], ones_row[:1, :], s_lo, start=False, stop=True)

                rec = work.tile([128, D], F32, tag="rec")
                nc.vector.reciprocal(rec[:], bd[:])
                y1 = work.tile([128, D], F32, tag="y1")
                nc.vector.tensor_mul(y1[:], bn[:], rec[:])
                y = work.tile([128, D], BF16, tag="y")
                nc.vector.tensor_mul(y[:], y1[:], sig[:])

                yT = transpose_chunk(y[:], 4, "yT")
                op = pj_ps.tile([128, D], F32, tag="pj")
                for j in range(4):
                    nc.tensor.matmul(
                        op[:], yT[:, ts(j, 128)], w_sb["wo"][:, j, :],
                        start=(j == 0), stop=False,
                    )
                nc.tensor.matmul(
                    op[:], ones_row[:1, :], b_sb["bo"][:1, :], start=False, stop=True
                )
                ot = work.tile([128, D], F32, tag="ot")
                nc.scalar.copy(ot[:], op[:])
                nc.sync.dma_start(out_d[ts(it, 128), :], ot[:])

    nc.finalize()
    return nc


def _shard_inputs(inputs):
    query = np.asarray(inputs["query"], np.float32)
    key = np.asarray(inputs["key"], np.float32)
    value = np.asarray(inputs["value"], np.float32)
    pos_bias = np.asarray(inputs["pos_bias"], np.float32)

    wT = {
        "wq": np.ascontiguousarray(np.asarray(inputs["Wq"], np.float32).T),
        "wk": np.ascontiguousarray(np.asarray(inputs["Wk"], np.float32).T),
        "wv": np.ascontiguousarray(np.asarray(inputs["Wv"], np.float32).T),
        "wo": np.ascontiguousarray(np.asarray(inputs["Wo"], np.float32).T),
    }
    bias = {
        "bq": np.asarray(inputs["bq"], np.float32).reshape(1, D),
        "bv": np.asarray(inputs["bv"], np.float32).reshape(1, D),
        "bo": np.asarray(inputs["bo"], np.float32).reshape(1, D),
    }
    # note: bk cancels in num/den and is deliberately unused.

    il = np.arange(128)[:, None]
    jl = np.arange(256)[None, :]
    band = ((jl > il) & (jl < il + 128)).astype(np.float32)

    szmask = np.zeros((128, 3), np.float32)
    szmask[64:, 0] = 1.0
    szmask[:, 1] = 1.0
    szmask[:64, 2] = 1.0

    in_maps = []
    for c in range(8):
        b, h = c // 2, c % 2
        r0 = h * TH
        q_s = np.ascontiguousarray(query[r0 : r0 + TH, b, :])

        lo, hi = r0 - WIN, r0 + TH + WIN
        k_s = np.zeros((KV, D), np.float32)
        v_s = np.zeros((KV, D), np.float32)
        slo, shi = max(lo, 0), min(hi, T)
        k_s[slo - lo : shi - lo] = key[slo:shi, b, :]
        v_s[slo - lo : shi - lo] = value[slo:shi, b, :]

        pb_s = np.zeros((NIT, 128, 256), np.float32)
        for it in range(NIT):
            i0 = r0 + it * 128
            j0 = i0 - WIN
            js, je = max(j0, 0), min(j0 + 256, T)
            pb_s[it, :, js - j0 : je - j0] = pos_bias[i0 : i0 + 128, js:je]
        pb_s *= band[None]

        m = {
            "q": q_s,
            "k": k_s,
            "v": v_s,
            "pb": np.ascontiguousarray(pb_s.reshape(NIT * 128, 256)),
            "szmask": szmask,
        }
        m.update(wT)
        m.update(bias)
        in_maps.append(m)
    return in_maps


def run(trace=False, **inputs):
    in_maps = _shard_inputs(inputs)
    nc = build()
    res = run_bass_kernel_spmd(nc, in_maps, core_ids=list(range(8)), trace=trace)
    out = np.zeros((T, B, D), np.float32)
    for c in range(8):
        b, h = c // 2, c % 2
        out[h * TH : (h + 1) * TH, b, :] = res.results[c]["out"]
    return out, res


def kernel(**inputs) -> np.ndarray:
    out, _ = run(trace=False, **inputs)
    return out


# revision 11
# speedup vs baseline: 1.2456x; 1.2456x over previous
"""AFT local-attention kernel for Trainium2 (8 NeuronCores).

Math (exactly equivalent to the reference in exact arithmetic):
  reference computes  y = sigmoid(q) * num / den  with
    num = einsum('ij,jbd->ibd', exp(pb*mask - max_pb), exp(k - max_key) * v)
    den = einsum('ij,jbd->ibd', exp(pb*mask - max_pb), exp(k - max_key))
  The row factor exp(-max_pb[i]) and the per-(b,d) factor exp(bk - max_key)
  are constant along the contraction axis j, so they cancel in num/den.
  Outside the |i-j| < 64 band, exp(pb*mask) == 1, so with
    E[i,j] = (exp(pb[i,j]) - 1) * band_mask[i,j]      (banded, width 127)
    S[b,d] = sum_j ek[j,b,d]*v[j,b,d],  Z[b,d] = sum_j ek[j,b,d]
    ek     = exp(key @ Wk.T)            (bk cancels too)
  we get  num = E @ (ek*v) + S,  den = E @ ek + Z.

Sharding: core c -> batch b=c//2, T-half h=c%2 (2048 queries each).
k/v are loaded with a 64-row halo (zero-padded at the global edges; the
band term is zero there because pb is zero-padded, and the halo rows are
excluded from the local S/Z partial sums). S/Z are all-reduced over core
pairs [2b, 2b+1].
"""

import numpy as np
from contextlib import ExitStack

import concourse.bass as bass
import concourse.tile as tile
from concourse import bacc, mybir
from concourse.bass import ts, ds
from concourse.bass_utils import run_bass_kernel_spmd
from concourse.masks import make_identity

T = 4096
B = 4
D = 512
WIN = 64
TH = T // 2          # tokens per core (query rows)
NIT = TH // 128      # 16 i-tiles
KV = TH + 2 * WIN    # 2176 k/v rows incl. halo
KCH = KV // 128      # 17 k/v chunks

F32 = mybir.dt.float32
BF16 = mybir.dt.bfloat16
AF = mybir.ActivationFunctionType


def build():
    nc = bacc.Bacc("TRN2", target_bir_lowering=False, num_devices=8)

    q_d = nc.dram_tensor("q", [TH, D], F32, kind="ExternalInput")
    k_d = nc.dram_tensor("k", [KV, D], F32, kind="ExternalInput")
    v_d = nc.dram_tensor("v", [KV, D], F32, kind="ExternalInput")
    pb_d = nc.dram_tensor("pb", [NIT * 128, 256], F32, kind="ExternalInput")
    w_d = {
        nm: nc.dram_tensor(nm, [D, D], F32, kind="ExternalInput")
        for nm in ("wq", "wk", "wv", "wo")
    }
    b_d = {
        nm: nc.dram_tensor(nm, [1, D], F32, kind="ExternalInput")
        for nm in ("bq", "bv", "bo")
    }
    szm_d = nc.dram_tensor("szmask", [128, 3], F32, kind="ExternalInput")
    out_d = nc.dram_tensor("out", [TH, D], F32, kind="ExternalOutput")

    with tile.TileContext(nc) as tc, ExitStack() as ctx:
        const = ctx.enter_context(tc.tile_pool(name="const", bufs=1))
        big = ctx.enter_context(tc.tile_pool(name="big", bufs=1))

        ident = const.tile([128, 128], BF16)
        make_identity(nc, ident[:])

        ones_row = const.tile([1, 128], BF16)
        nc.vector.memset(ones_row[:], 1.0)

        szm = const.tile([128, 3], BF16)
        nc.gpsimd.dma_start(szm[:], szm_d[:, :])

        w_sb = {}
        for nm in ("wq", "wk", "wv", "wo"):
            wt = const.tile([128, 4, D], BF16, tag=f"w_{nm}")
            nc.gpsimd.dma_start(wt[:], w_d[nm][:, :].rearrange("(c p) n -> p c n", p=128))
            w_sb[nm] = wt
        b_sb = {}
        for nm in ("bq", "bv", "bo"):
            bt = const.tile([1, D], BF16, tag=f"b_{nm}")
            nc.gpsimd.dma_start(bt[:], b_d[nm][:, :])
            b_sb[nm] = bt

        # natural-layout (token-major) staging for the three inputs
        k_nat = big.tile([128, KCH, D], BF16)
        v_nat = big.tile([128, KCH, D], BF16)
        q_nat = big.tile([128, NIT, D], BF16)
        pb_nat = big.tile([128, NIT, 256], BF16)
        for c0 in range(0, KCH, 5):
            cn = min(5, KCH - c0)
            nc.gpsimd.dma_start(
                k_nat[:, c0 : c0 + cn, :],
                k_d[:, :].rearrange("(c p) n -> p c n", p=128)[:, c0 : c0 + cn, :],
            )
            nc.gpsimd.dma_start(
                v_nat[:, c0 : c0 + cn, :],
                v_d[:, :].rearrange("(c p) n -> p c n", p=128)[:, c0 : c0 + cn, :],
            )
        for c0 in range(0, NIT, 4):
            nc.gpsimd.dma_start(
                q_nat[:, c0 : c0 + 4, :],
                q_d[:, :].rearrange("(c p) n -> p c n", p=128)[:, c0 : c0 + 4, :],
            )
        nc.gpsimd.dma_start(
            pb_nat[:], pb_d[:, :].rearrange("(c p) n -> p c n", p=128)
        )

        ek_big = big.tile([128, KCH * D], BF16)
        ekv_big = big.tile([128, KCH * D], BF16)
        et_big = big.tile([128, NIT * 256], BF16)

        lhsT_p = ctx.enter_context(tc.tile_pool(name="lhsT", bufs=3))
        tp_ps = ctx.enter_context(tc.tile_pool(name="tpps", bufs=2, space="PSUM"))
        pj_ps = ctx.enter_context(tc.tile_pool(name="pjps", bufs=2, space="PSUM"))
        work = ctx.enter_context(tc.tile_pool(name="work", bufs=3))
        dram = ctx.enter_context(tc.tile_pool(name="dram", bufs=1, space="DRAM"))

        def transpose_chunk(src_ap, nq, tag):
            """PE-transpose a [128, nq*128] natural chunk -> [128, nq*128] where
            quarter j holds src[:, j*128:(j+1)*128].T; returns an SBUF bf16 tile."""
            ps = tp_ps.tile([128, nq * 128], BF16, tag="tp")
            for j in range(nq):
                nc.tensor.transpose(ps[:, ts(j, 128)], src_ap[:, ts(j, 128)], ident[:])
            sb = lhsT_p.tile([128, nq * 128], BF16, tag=tag)
            nc.vector.tensor_copy(sb[:], ps[:])
            return sb

        # ---- phase 1: k/v projections, ek/ekv, S/Z partial sums ----
        with tc.tile_pool(name="szps", bufs=1, space="PSUM") as szp:
            z_ps = szp.tile([1, D], F32, tag="z")
            s_ps = szp.tile([1, D], F32, tag="s")
            for c in range(KCH):
                sel = 0 if c == 0 else (2 if c == KCH - 1 else 1)
                kT = transpose_chunk(k_nat[:, c, :], 4, "xT")
                kp = pj_ps.tile([128, D], F32, tag="pj")
                for j in range(4):
                    nc.tensor.matmul(
                        kp[:], kT[:, ts(j, 128)], w_sb["wk"][:, j, :],
                        start=(j == 0), stop=(j == 3),
                    )
                nc.scalar.activation(ek_big[:, ts(c, D)], kp[:], AF.Exp)

                vT = transpose_chunk(v_nat[:, c, :], 4, "xT")
                vp = pj_ps.tile([128, D], F32, tag="pj")
                for j in range(4):
                    nc.tensor.matmul(
                        vp[:], vT[:, ts(j, 128)], w_sb["wv"][:, j, :],
                        start=(j == 0), stop=False,
                    )
                nc.tensor.matmul(
                    vp[:], ones_row[:1, :], b_sb["bv"][:1, :], start=False, stop=True
                )
                nc.vector.tensor_mul(ekv_big[:, ts(c, D)], ek_big[:, ts(c, D)], vp[:])

                nc.tensor.matmul(
                    z_ps[:], szm[:, sel : sel + 1], ek_big[:, ts(c, D)],
                    start=(c == 0), stop=(c == KCH - 1), skip_group_check=True,
                )
                nc.tensor.matmul(
                    s_ps[:], szm[:, sel : sel + 1], ekv_big[:, ts(c, D)],
                    start=(c == 0), stop=(c == KCH - 1), skip_group_check=True,
                )

            # ---- phase 2: S/Z all-reduce over the core pair ----
            sz_sb = const.tile([1, 2 * D], F32)
            nc.vector.tensor_copy(sz_sb[:, 0:D], z_ps[:])
            nc.vector.tensor_copy(sz_sb[:, D : 2 * D], s_ps[:])

        cc_in = dram.tile([1, 2 * D], F32)
        cc_out = dram.tile([1, 2 * D], F32)  # Shared addr_space unsupported for 2-core groups
        nc.sync.dma_start(cc_in[:], sz_sb[:])
        nc.gpsimd.collective_compute(
            "AllReduce",
            mybir.AluOpType.add,
            replica_groups=[[0, 1], [2, 3], [4, 5], [6, 7]],
            ins=[cc_in[:].opt()],
            outs=[cc_out[:].opt()],
        )
        # broadcast Z|S to all 128 partitions (DMA only — keeps the PE stream
        # free of any dependency on the collective)
        zbsb = const.tile([128, 2 * D], F32)
        nc.sync.dma_start(zbsb[:], cc_out[:].partition_broadcast(128))
        ZB = zbsb[:, 0:D]
        SB = zbsb[:, D : 2 * D]

        # ---- phase 3: E.T tiles from pos_bias band slices ----
        for it in range(NIT):
            pps = tp_ps.tile([128, 256], BF16, tag="tp")
            for j in range(2):
                nc.tensor.transpose(
                    pps[:, ts(j, 128)], pb_nat[:, it, ts(j, 128)], ident[:]
                )
            pe_f = work.tile([128, 256], F32, tag="pexp")
            nc.scalar.activation(pe_f[:], pps[:], AF.Exp)
            nc.vector.tensor_scalar_sub(et_big[:, ts(it, 256)], pe_f[:], 1.0)

        # ---- phase 4a: all q projections + sigmoids (fills the all-reduce
        # latency window with PE work that doesn't depend on it) ----
        sig_tiles = []
        for it in range(NIT):
            qT = transpose_chunk(q_nat[:, it, :], 4, "xT")
            qp = pj_ps.tile([128, D], F32, tag="pj")
            for j in range(4):
                nc.tensor.matmul(
                    qp[:], qT[:, ts(j, 128)], w_sb["wq"][:, j, :],
                    start=(j == 0), stop=False,
                )
            nc.tensor.matmul(
                qp[:], ones_row[:1, :], b_sb["bq"][:1, :], start=False, stop=True
            )
            sig = const.tile([128, D], F32, tag=f"sig{it}")
            nc.scalar.activation(sig[:], qp[:], AF.Sigmoid)
            sig_tiles.append(sig)

        # ---- phase 4b: per i-tile band einsum, combine, out projection ----
        with tc.tile_pool(name="einps", bufs=2, space="PSUM") as einp:
            for it in range(NIT):
                sig = sig_tiles[it]
                bn = einp.tile([128, D], F32, tag="bn")
                bd = einp.tile([128, D], F32, tag="bd")
                e0 = et_big[:, ds(it * 256, 128)]
                e1 = et_big[:, ds(it * 256 + 128, 128)]
                nc.tensor.matmul(bd[:], e0, ek_big[:, ts(it, D)], start=True, stop=False)
                nc.tensor.matmul(bd[:], e1, ek_big[:, ts(it + 1, D)], start=False, stop=True)
                nc.tensor.matmul(bn[:], e0, ekv_big[:, ts(it, D)], start=True, stop=False)
                nc.tensor.matmul(bn[:], e1, ekv_big[:, ts(it + 1, D)], start=False, stop=True)

                den = work.tile([128, D], F32, tag="den")
                nc.vector.tensor_tensor(den[:], bd[:], ZB, mybir.AluOpType.add)
                num = work.tile([128, D], F32, tag="num")
                nc.vector.tensor_tensor(num[:], bn[:], SB, mybir.AluOpType.add)
                rec = work.tile([128, D], F32, tag="rec")
                nc.vector.reciprocal_approx_fast(rec[:], den[:])
                y1 = work.tile([128, D], F32, tag="y1")
                nc.vector.tensor_mul(y1[:], num[:], rec[:])
                y = work.tile([128, D], BF16, tag="y")
                nc.gpsimd.tensor_tensor(y[:], y1[:], sig[:], mybir.AluOpType.mult)

                yT = transpose_chunk(y[:], 4, "yT")
                op = pj_ps.tile([128, D], F32, tag="pj")
                for j in range(4):
                    nc.tensor.matmul(
                        op[:], yT[:, ts(j, 128)], w_sb["wo"][:, j, :],
                        start=(j == 0), stop=False,
                    )
                nc.tensor.matmul(
                    op[:], ones_row[:1, :], b_sb["bo"][:1, :], start=False, stop=True
                )
                ot = work.tile([128, D], F32, tag="ot")
                nc.scalar.copy(ot[:], op[:])
                nc.sync.dma_start(out_d[ts(it, 128), :], ot[:])

    nc.finalize()
    return nc


def _shard_inputs(inputs):
    query = np.asarray(inputs["query"], np.float32)
    key = np.asarray(inputs["key"], np.float32)
    value = np.asarray(inputs["value"], np.float32)
    pos_bias = np.asarray(inputs["pos_bias"], np.float32)

    wT = {
        "wq": np.ascontiguousarray(np.asarray(inputs["Wq"], np.float32).T),
        "wk": np.ascontiguousarray(np.asarray(inputs["Wk"], np.float32).T),
        "wv": np.ascontiguousarray(np.asarray(inputs["Wv"], np.float32).T),
        "wo": np.ascontiguousarray(np.asarray(inputs["Wo"], np.float32).T),
    }
    bias = {
        "bq": np.asarray(inputs["bq"], np.float32).reshape(1, D),
        "bv": np.asarray(inputs["bv"], np.float32).reshape(1, D),
        "bo": np.asarray(inputs["bo"], np.float32).reshape(1, D),
    }
    # note: bk cancels in num/den and is deliberately unused.

    il = np.arange(128)[:, None]
    jl = np.arange(256)[None, :]
    band = ((jl > il) & (jl < il + 128)).astype(np.float32)

    szmask = np.zeros((128, 3), np.float32)
    szmask[64:, 0] = 1.0
    szmask[:, 1] = 1.0
    szmask[:64, 2] = 1.0

    in_maps = []
    for c in range(8):
        b, h = c // 2, c % 2
        r0 = h * TH
        q_s = np.ascontiguousarray(query[r0 : r0 + TH, b, :])

        lo, hi = r0 - WIN, r0 + TH + WIN
        k_s = np.zeros((KV, D), np.float32)
        v_s = np.zeros((KV, D), np.float32)
        slo, shi = max(lo, 0), min(hi, T)
        k_s[slo - lo : shi - lo] = key[slo:shi, b, :]
        v_s[slo - lo : shi - lo] = value[slo:shi, b, :]

        pb_s = np.zeros((NIT, 128, 256), np.float32)
        for it in range(NIT):
            i0 = r0 + it * 128
            j0 = i0 - WIN
            js, je = max(j0, 0), min(j0 + 256, T)
            pb_s[it, :, js - j0 : je - j0] = pos_bias[i0 : i0 + 128, js:je]
        pb_s *= band[None]

        m = {
            "q": q_s,
            "k": k_s,
            "v": v_s,
            "pb": np.ascontiguousarray(pb_s.reshape(NIT * 128, 256)),
            "szmask": szmask,
        }
        m.update(wT)
        m.update(bias)
        in_maps.append(m)
    return in_maps


def run(trace=False, **inputs):
    in_maps = _shard_inputs(inputs)
    nc = build()
    res = run_bass_kernel_spmd(nc, in_maps, core_ids=list(range(8)), trace=trace)
    out = np.zeros((T, B, D), np.float32)
    for c in range(8):
        b, h = c // 2, c % 2
        out[h * TH : (h + 1) * TH, b, :] = res.results[c]["out"]
    return out, res


def kernel(**inputs) -> np.ndarray:
    out, _ = run(trace=False, **inputs)
    return out
